# revision 5
# baseline (speedup 1.0000x reference)
"""Multi-head causal attention (B=2, T=2048, E=1024, H=16, D=64) on 8 TRN2 cores.

Sharding: tensor-parallel over heads. Core c owns heads {2c, 2c+1} for both
batches. Each core computes its heads' q/k/v projections, causal attention,
and a partial output projection z_c = out_c @ Wo[:, 128c:128c+128].T.
Host combines: z = sum_c z_c + bo.

Note the reference computes wei = K @ Q^T, i.e. output token t attends over
s <= t with logits k_t . q_s. We compute ST[s, t] = q_s . k_t (s on
partitions) so that the A@V matmul needs no transposes, and get the softmax
denominator via a ones-column appended to V.
"""

import numpy as np
import ml_dtypes

import concourse.bacc as bacc
import concourse.mybir as mybir
import concourse.tile as tile
from concourse.bass_utils import run_bass_kernel_spmd
from concourse.masks import make_identity

N_CORES = 8
B, T, E = 2, 2048, 1024
H, D = 16, 64
HPC = H // N_CORES          # heads per core = 2
F = HPC * D                 # local feature cols = 128
TBLK = 512                  # t-block width for stage A
NTB = T // TBLK             # 4
NSC = T // 128              # s-chunks = 16
NEC = E // 128              # e-chunks = 8
EXP_BIAS = -2.0             # exp(S + EXP_BIAS); cancels in softmax, guards overflow

F32 = mybir.dt.float32
F32R = mybir.dt.float32r
BF16 = mybir.dt.bfloat16
EXP = mybir.ActivationFunctionType.Exp


def build_nc(rep=1):
    nc = bacc.Bacc("TRN2", target_bir_lowering=False, debug=False,
                   num_devices=N_CORES)

    xt = nc.dram_tensor("xt", [B, E, T], F32R, kind="ExternalInput").ap()
    wq = nc.dram_tensor("wq", [E, F], F32R, kind="ExternalInput").ap()
    wk = nc.dram_tensor("wk", [E, F], F32R, kind="ExternalInput").ap()
    wv = nc.dram_tensor("wv", [E, F], F32R, kind="ExternalInput").ap()
    wot = nc.dram_tensor("wot", [F, E], F32R, kind="ExternalInput").ap()
    mask = nc.dram_tensor("mask", [128, 128], BF16, kind="ExternalInput").ap()
    zp = nc.dram_tensor("zp", [B, T, E], F32, kind="ExternalOutput").ap()

    with tile.TileContext(nc) as tc:
        with (
            tc.tile_pool(name="const", bufs=1) as cpool,
            tc.tile_pool(name="xtp", bufs=9) as xtp,
            tc.tile_pool(name="proj", bufs=2) as projp,
            tc.tile_pool(name="v2p", bufs=2 * NSC) as v2p,
            tc.tile_pool(name="ptp", bufs=4) as ptp,
            tc.tile_pool(name="smallp", bufs=4) as smallp,
            tc.tile_pool(name="zsbp", bufs=3) as zsbp,
            tc.tile_pool(name="ps_s", bufs=2, space="PSUM") as ps_s,
            tc.tile_pool(name="ps_o", bufs=2, space="PSUM") as ps_o,
            tc.tile_pool(name="ps_t", bufs=1, space="PSUM") as ps_t,
            tc.tile_pool(name="ps_z", bufs=1, space="PSUM") as ps_z,
        ):
            # ---- constants (loaded once) ----
            ident = cpool.tile([128, 128], F32, tag="ident")
            make_identity(nc, ident[:])
            mask_sb = cpool.tile([128, 128], BF16, tag="mask")
            nc.sync.dma_start(mask_sb[:], mask)
            zrow = cpool.tile([1, 260], F32, tag="zrow")
            nc.vector.memset(zrow[:], 0.0)
            zcol = cpool.tile([1, 128], F32, tag="zcol")
            nc.vector.memset(zcol[:], 0.0)
            ebias = cpool.tile([128, 1], F32, tag="ebias")
            nc.vector.memset(ebias[:], EXP_BIAS)
            wq_sb = []
            wk_sb = []
            wv_sb = []
            for e in range(NEC):
                for lst, src, nm in ((wq_sb, wq, "wq"), (wk_sb, wk, "wk"),
                                     (wv_sb, wv, "wv")):
                    t_ = cpool.tile([128, F], F32R, tag=f"{nm}{e}")
                    nc.sync.dma_start(t_[:], src[e * 128:(e + 1) * 128, :])
                    lst.append(t_)
            wot_sb = cpool.tile([F, E], F32R, tag="wot")
            nc.sync.dma_start(wot_sb[:], wot)

            def body():
                for b in range(B):
                    # ---- load transposed activations ----
                    xts = []
                    for e in range(NEC):
                        t_ = xtp.tile([128, T], F32R, tag="xt")
                        nc.sync.dma_start(t_[:], xt[b, e * 128:(e + 1) * 128, :])
                        xts.append(t_)

                    # ---- projections: qT2/kT2/vT2 [128(f), T] ----
                    heads = {}
                    for nm, wsb in (("q", wq_sb), ("k", wk_sb), ("v", wv_sb)):
                        dst = projp.tile([128, T], F32R if nm != "v" else F32, tag=f"{nm}T2")
                        for tp2 in range(T // 1024):
                            ps = ps_s.tile([128, 1024], F32, tag="sp")
                            for half in range(2):
                                c0 = tp2 * 1024 + half * 512
                                for e in range(NEC):
                                    nc.tensor.matmul(
                                        ps[:, half * 512:(half + 1) * 512],
                                        wsb[e][:],
                                        xts[e][:, c0:c0 + 512],
                                        start=(e == 0), stop=(e == NEC - 1))
                            nc.vector.tensor_copy(
                                dst[:, tp2 * 1024:(tp2 + 1) * 1024], ps[:])
                        heads[nm] = dst
                    qT2, kT2, vT2 = heads["q"], heads["k"], heads["v"]

                    # ---- v2[s]: [128(s), 130] bf16 = [1|v_h0|1|v_h1] ----
                    v2 = []
                    for s in range(NSC):
                        tp_ = ps_t.tile([128, 128], F32, tag="tp")
                        nc.tensor.matmul(tp_[:], vT2[:, s * 128:(s + 1) * 128],
                                         ident[:], is_transpose=True)
                        v2t = v2p.tile([128, 130], BF16, tag="v2")
                        v2r = v2t.rearrange("p (g c) -> p g c", g=2)
                        nc.vector.memset(v2r[:, :, 0:1], 1.0)
                        nc.vector.tensor_copy(
                            v2r[:, :, 1:65],
                            tp_.rearrange("p (g c) -> p g c", g=2))
                        v2.append(v2t)

                    # ---- attention ----
                    for tb in range(NTB):
                        po = []
                        for jp in range(2):
                            p_ = ps_o.tile([128, 260], F32, tag="op")
                            # zero-fill so stage-B matmuls can accumulate in
                            # any order without a bank-clearing start matmul
                            nc.tensor.matmul(p_[:], zcol[:],
                                             zrow[:],
                                             start=True, stop=True)
                            po.append(p_)
                        npairs = 2 * tb + 2
                        for p in range(npairs):
                            pts = []
                            for h in range(2):
                                ps = ps_s.tile([128, 1024], F32, tag="sp")
                                for dp in range(2):
                                    si = 2 * p + dp
                                    nc.tensor.matmul(
                                        ps[:, dp * 512:(dp + 1) * 512],
                                        qT2[64 * h:64 * h + 64,
                                            si * 128:(si + 1) * 128],
                                        kT2[64 * h:64 * h + 64,
                                            tb * 512:(tb + 1) * 512],
                                        start=True, stop=True)
                                pt = ptp.tile([128, 1024], BF16, tag="pt")
                                nc.scalar.activation(pt[:], ps[:], EXP,
                                                     bias=ebias[:])
                                for dp in range(2):
                                    si = 2 * p + dp
                                    r = si - 4 * tb
                                    if 0 <= r < 4:
                                        sl = pt[:, dp * 512 + r * 128:
                                                dp * 512 + (r + 1) * 128]
                                        nc.vector.tensor_mul(sl, sl, mask_sb[:])
                                pts.append(pt)
                            for dp in range(2):
                                si = 2 * p + dp
                                for h in range(2):
                                    for j in range(4):
                                        tcg = 4 * tb + j
                                        if si > tcg:
                                            continue
                                        jp, jj = j // 2, j % 2
                                        nc.tensor.matmul(
                                            po[jp][:, jj * 130 + h * 65:
                                                   jj * 130 + (h + 1) * 65],
                                            pts[h][:, dp * 512 + j * 128:
                                                   dp * 512 + (j + 1) * 128],
                                            v2[si][:, h * 65:(h + 1) * 65],
                                            start=False, stop=(si == tcg),
                                            skip_group_check=True)

                        # ---- normalize + transpose + partial z ----
                        for jp in range(2):
                            rinv = smallp.tile([128, 4], F32, tag="rinv")
                            lv = po[jp].rearrange("p (g c) -> p g c", g=4)
                            nc.vector.reciprocal(
                                rinv.rearrange("p (g c) -> p g c", c=1),
                                lv[:, :, 0:1])
                            for jj in range(2):
                                j = 2 * jp + jj
                                tcg = 4 * tb + j
                                out2 = smallp.tile([128, 128], F32, tag="out2")
                                for h in range(2):
                                    nc.vector.tensor_scalar_mul(
                                        out2[:, h * 64:(h + 1) * 64],
                                        po[jp][:, jj * 130 + h * 65 + 1:
                                               jj * 130 + h * 65 + 65],
                                        rinv[:, 2 * jj + h:2 * jj + h + 1])
                                tp_ = ps_t.tile([128, 128], F32, tag="tp")
                                nc.tensor.matmul(tp_[:], out2[:], ident[:],
                                                 is_transpose=True)
                                outT = smallp.tile([128, 128], F32R, tag="outT")
                                nc.vector.tensor_copy(outT[:], tp_[:])
                                zsb = zsbp.tile([128, 1024], F32, tag="zsb")
                                for eb in range(2):
                                    zps = ps_z.tile([128, 512], F32, tag="zp")
                                    nc.tensor.matmul(
                                        zps[:], outT[:],
                                        wot_sb[:, eb * 512:(eb + 1) * 512]
                                        ,
                                        start=True, stop=True)
                                    nc.vector.tensor_copy(
                                        zsb[:, eb * 512:(eb + 1) * 512], zps[:])
                                nc.sync.dma_start(
                                    zp[b, tcg * 128:(tcg + 1) * 128, :], zsb[:])

            if rep == 1:
                body()
            else:
                with tc.For_i(0, rep, 1):
                    body()

    nc.compile()
    return nc


def make_in_maps(inputs, Wk, Wq, Wv, Wo):
    """Shard full inputs into per-core input maps."""
    xt = np.ascontiguousarray(inputs.transpose(0, 2, 1)).astype(np.float32)
    scale = np.float32(D ** -0.5)
    tri = (np.arange(128)[:, None] <= np.arange(128)[None, :])
    mask = tri.astype(ml_dtypes.bfloat16)
    in_maps = []
    for c in range(N_CORES):
        h0 = HPC * c
        wq2 = np.ascontiguousarray(
            np.concatenate([Wq[h0 + i] for i in range(HPC)], axis=1))
        wk2 = np.ascontiguousarray(
            np.concatenate([Wk[h0 + i] for i in range(HPC)], axis=1)) * scale
        wv2 = np.ascontiguousarray(
            np.concatenate([Wv[h0 + i] for i in range(HPC)], axis=1))
        wot = np.ascontiguousarray(Wo[:, F * c:F * (c + 1)].T)
        in_maps.append({
            "xt": xt,
            "wq": wq2.astype(np.float32),
            "wk": wk2.astype(np.float32),
            "wv": wv2.astype(np.float32),
            "wot": wot.astype(np.float32),
            "mask": mask,
        })
    return in_maps


_NC = None


def kernel(inputs, Wk, Wq, Wv, Wo, bo):
    global _NC
    if _NC is None:
        _NC = build_nc()
    in_maps = make_in_maps(inputs, Wk, Wq, Wv, Wo)
    res = run_bass_kernel_spmd(_NC, in_maps, core_ids=list(range(N_CORES)))
    z = np.zeros((B, T, E), dtype=np.float32)
    for c in range(N_CORES):
        z += res.results[c]["zp"]
    return z + bo.astype(np.float32)


# revision 8
# speedup vs baseline: 1.1942x; 1.1942x over previous
"""Multi-head causal attention (B=2, T=2048, E=1024, H=16, D=64) on 8 TRN2 cores.

Sharding: tensor-parallel over heads. Core c owns heads {2c, 2c+1} for both
batches. Each core computes its heads' q/k/v projections, causal attention,
and a partial output projection z_c = out_c @ Wo[:, 128c:128c+128].T.
Host combines: z = sum_c z_c + bo.

Note the reference computes wei = K @ Q^T, i.e. output token t attends over
s <= t with logits k_t . q_s. We compute ST[s, t] = q_s . k_t (s on
partitions) so that the A@V matmul needs no transposes, and get the softmax
denominator via a ones-column appended to V.
"""

import numpy as np
import ml_dtypes

import concourse.bacc as bacc
import concourse.mybir as mybir
import concourse.tile as tile
from concourse.bass_utils import run_bass_kernel_spmd
from concourse.masks import make_identity

N_CORES = 8
B, T, E = 2, 2048, 1024
H, D = 16, 64
HPC = H // N_CORES          # heads per core = 2
F = HPC * D                 # local feature cols = 128
TBLK = 512                  # t-block width for stage A
NTB = T // TBLK             # 4
NSC = T // 128              # s-chunks = 16
NEC = E // 128              # e-chunks = 8
EXP_BIAS = -2.0             # exp(S + EXP_BIAS); cancels in softmax, guards overflow

F32 = mybir.dt.float32
F16 = mybir.dt.float16
F32R = mybir.dt.float32r
BF16 = mybir.dt.bfloat16
EXP = mybir.ActivationFunctionType.Exp


def build_nc(rep=1, cfg=None):
    cfg = dict(cfg or {})
    any_copy = cfg.get("any_copy", False)
    sp_bufs = cfg.get("sp_bufs", 2)
    op_bufs = cfg.get("op_bufs", 2)
    misc_bufs = cfg.get("misc_bufs", None)  # if set, tp+zp merged [128,512] x misc_bufs
    pt_bufs = cfg.get("pt_bufs", 4)
    nc = bacc.Bacc("TRN2", target_bir_lowering=False, debug=False,
                   num_devices=N_CORES)

    xt = nc.dram_tensor("xt", [B, E, T], F32R, kind="ExternalInput").ap()
    wq = nc.dram_tensor("wq", [E, F], F32R, kind="ExternalInput").ap()
    wk = nc.dram_tensor("wk", [E, F], F32R, kind="ExternalInput").ap()
    wv = nc.dram_tensor("wv", [E, F], F32R, kind="ExternalInput").ap()
    wot = nc.dram_tensor("wot", [F, E], F32R, kind="ExternalInput").ap()
    mask = nc.dram_tensor("mask", [128, 128], BF16, kind="ExternalInput").ap()
    zp = nc.dram_tensor("zp", [B, T, E], F16, kind="ExternalOutput").ap()

    with tile.TileContext(nc) as tc:
        with (
            tc.tile_pool(name="const", bufs=1) as cpool,
            tc.tile_pool(name="xtp", bufs=9) as xtp,
            tc.tile_pool(name="proj", bufs=2) as projp,
            tc.tile_pool(name="v2p", bufs=2 * NSC) as v2p,
            tc.tile_pool(name="ptp", bufs=pt_bufs) as ptp,
            tc.tile_pool(name="smallp", bufs=4) as smallp,
            tc.tile_pool(name="zsbp", bufs=3) as zsbp,
            tc.tile_pool(name="ps_s", bufs=sp_bufs, space="PSUM") as ps_s,
            tc.tile_pool(name="ps_o", bufs=op_bufs, space="PSUM") as ps_o,
            tc.tile_pool(name="ps_t", bufs=(misc_bufs or 1), space="PSUM") as ps_t,
            tc.tile_pool(name="ps_z", bufs=(0 if misc_bufs else 1) or 1, space="PSUM") as ps_z,
        ):
            # ---- constants (loaded once) ----
            ident = cpool.tile([128, 128], F32, tag="ident")
            make_identity(nc, ident[:])
            mask_sb = cpool.tile([128, 128], BF16, tag="mask")
            nc.sync.dma_start(mask_sb[:], mask)
            zrow = cpool.tile([1, 260], BF16, tag="zrow")
            nc.vector.memset(zrow[:], 0.0)
            zcol = cpool.tile([1, 128], BF16, tag="zcol")
            nc.vector.memset(zcol[:], 0.0)
            ebias = cpool.tile([128, 1], F32, tag="ebias")
            nc.vector.memset(ebias[:], EXP_BIAS)
            wq_sb = []
            wk_sb = []
            wv_sb = []
            for e in range(NEC):
                for lst, src, nm in ((wq_sb, wq, "wq"), (wk_sb, wk, "wk"),
                                     (wv_sb, wv, "wv")):
                    t_ = cpool.tile([128, F], F32R, tag=f"{nm}{e}")
                    nc.sync.dma_start(t_[:], src[e * 128:(e + 1) * 128, :])
                    lst.append(t_)
            wot_sb = cpool.tile([F, E], F32R, tag="wot")
            nc.sync.dma_start(wot_sb[:], wot)

            def body():
                for b in range(B):
                    # ---- load transposed activations ----
                    xts = []
                    for e in range(NEC):
                        t_ = xtp.tile([128, T], F32R, tag="xt")
                        nc.sync.dma_start(t_[:], xt[b, e * 128:(e + 1) * 128, :])
                        xts.append(t_)

                    # ---- projections: qT2/kT2/vT2 [128(f), T] ----
                    heads = {}
                    for nm, wsb in (("q", wq_sb), ("k", wk_sb), ("v", wv_sb)):
                        dst = projp.tile([128, T], F32R if nm != "v" else F32, tag=f"{nm}T2")
                        for tp2 in range(T // 1024):
                            ps = ps_s.tile([128, 1024], F32, tag="sp")
                            for half in range(2):
                                c0 = tp2 * 1024 + half * 512
                                for e in range(NEC):
                                    nc.tensor.matmul(
                                        ps[:, half * 512:(half + 1) * 512],
                                        wsb[e][:],
                                        xts[e][:, c0:c0 + 512],
                                        start=(e == 0), stop=(e == NEC - 1))
                            (nc.any if any_copy else nc.vector).tensor_copy(
                                dst[:, tp2 * 1024:(tp2 + 1) * 1024], ps[:])
                        heads[nm] = dst
                    qT2, kT2, vT2 = heads["q"], heads["k"], heads["v"]

                    # ---- v2[s]: [128(s), 130] bf16 = [1|v_h0|1|v_h1] ----
                    v2 = []
                    for s in range(NSC):
                        if misc_bufs:
                            tpw = ps_t.tile([128, 512], F32, tag="tp")
                            tp_ = tpw[:, 0:128]
                        else:
                            tp_ = ps_t.tile([128, 128], F32, tag="tp")
                        nc.tensor.matmul(tp_[:], vT2[:, s * 128:(s + 1) * 128],
                                         ident[:], is_transpose=True)
                        v2t = v2p.tile([128, 130], BF16, tag="v2")
                        v2r = v2t.rearrange("p (g c) -> p g c", g=2)
                        nc.vector.memset(v2r[:, :, 0:1], 1.0)
                        nc.vector.tensor_copy(
                            v2r[:, :, 1:65],
                            tp_.rearrange("p (g c) -> p g c", g=2))
                        v2.append(v2t)

                    # ---- attention ----
                    for tb in range(NTB):
                        po = []
                        for jp in range(2):
                            p_ = ps_o.tile([128, 260], F32, tag="op")
                            # zero-fill so stage-B matmuls can accumulate in
                            # any order without a bank-clearing start matmul
                            nc.tensor.matmul(p_[:], zcol[:],
                                             zrow[:],
                                             start=True, stop=True)
                            po.append(p_)
                        npairs = 2 * tb + 2
                        for p in range(npairs):
                            pts = []
                            for h in range(2):
                                ps = ps_s.tile([128, 1024], F32, tag="sp")
                                for dp in range(2):
                                    si = 2 * p + dp
                                    nc.tensor.matmul(
                                        ps[:, dp * 512:(dp + 1) * 512],
                                        qT2[64 * h:64 * h + 64,
                                            si * 128:(si + 1) * 128],
                                        kT2[64 * h:64 * h + 64,
                                            tb * 512:(tb + 1) * 512],
                                        start=True, stop=True)
                                pt = ptp.tile([128, 1024], BF16, tag="pt")
                                nc.scalar.activation(pt[:], ps[:], EXP,
                                                     bias=ebias[:])
                                for dp in range(2):
                                    si = 2 * p + dp
                                    r = si - 4 * tb
                                    if 0 <= r < 4:
                                        sl = pt[:, dp * 512 + r * 128:
                                                dp * 512 + (r + 1) * 128]
                                        nc.vector.tensor_mul(sl, sl, mask_sb[:])
                                pts.append(pt)
                            for dp in range(2):
                                si = 2 * p + dp
                                for h in range(2):
                                    for j in range(4):
                                        tcg = 4 * tb + j
                                        if si > tcg:
                                            continue
                                        jp, jj = j // 2, j % 2
                                        nc.tensor.matmul(
                                            po[jp][:, jj * 130 + h * 65:
                                                   jj * 130 + (h + 1) * 65],
                                            pts[h][:, dp * 512 + j * 128:
                                                   dp * 512 + (j + 1) * 128],
                                            v2[si][:, h * 65:(h + 1) * 65],
                                            start=False, stop=(si == tcg),
                                            skip_group_check=True)

                        # ---- normalize + transpose + partial z ----
                        for jp in range(2):
                            rinv = smallp.tile([128, 4], F32, tag="rinv")
                            lv = po[jp].rearrange("p (g c) -> p g c", g=4)
                            nc.vector.reciprocal(
                                rinv.rearrange("p (g c) -> p g c", c=1),
                                lv[:, :, 0:1])
                            for jj in range(2):
                                j = 2 * jp + jj
                                tcg = 4 * tb + j
                                out2 = smallp.tile([128, 128], F32, tag="out2")
                                for h in range(2):
                                    nc.vector.tensor_scalar_mul(
                                        out2[:, h * 64:(h + 1) * 64],
                                        po[jp][:, jj * 130 + h * 65 + 1:
                                               jj * 130 + h * 65 + 65],
                                        rinv[:, 2 * jj + h:2 * jj + h + 1])
                                if misc_bufs:
                                    tpw = ps_t.tile([128, 512], F32, tag="tp")
                                    tp_ = tpw[:, 0:128]
                                else:
                                    tp_ = ps_t.tile([128, 128], F32, tag="tp")
                                nc.tensor.matmul(tp_[:], out2[:], ident[:],
                                                 is_transpose=True)
                                outT = smallp.tile([128, 128], F32R, tag="outT")
                                (nc.any if any_copy else nc.vector).tensor_copy(outT[:], tp_[:])
                                zsb = zsbp.tile([128, 1024], F16, tag="zsb")
                                for eb in range(2):
                                    if misc_bufs:
                                        zps = ps_t.tile([128, 512], F32, tag="tp")
                                    else:
                                        zps = ps_z.tile([128, 512], F32, tag="zp")
                                    nc.tensor.matmul(
                                        zps[:], outT[:],
                                        wot_sb[:, eb * 512:(eb + 1) * 512]
                                        ,
                                        start=True, stop=True)
                                    (nc.any if any_copy else nc.vector).tensor_copy(
                                        zsb[:, eb * 512:(eb + 1) * 512], zps[:])
                                nc.sync.dma_start(
                                    zp[b, tcg * 128:(tcg + 1) * 128, :], zsb[:])

            if rep == 1:
                body()
            else:
                with tc.For_i(0, rep, 1):
                    body()

    nc.compile()
    return nc


def make_in_maps(inputs, Wk, Wq, Wv, Wo):
    """Shard full inputs into per-core input maps."""
    xt = np.ascontiguousarray(inputs.transpose(0, 2, 1)).astype(np.float32)
    scale = np.float32(D ** -0.5)
    tri = (np.arange(128)[:, None] <= np.arange(128)[None, :])
    mask = tri.astype(ml_dtypes.bfloat16)
    in_maps = []
    for c in range(N_CORES):
        h0 = HPC * c
        wq2 = np.ascontiguousarray(
            np.concatenate([Wq[h0 + i] for i in range(HPC)], axis=1))
        wk2 = np.ascontiguousarray(
            np.concatenate([Wk[h0 + i] for i in range(HPC)], axis=1)) * scale
        wv2 = np.ascontiguousarray(
            np.concatenate([Wv[h0 + i] for i in range(HPC)], axis=1))
        wot = np.ascontiguousarray(Wo[:, F * c:F * (c + 1)].T)
        in_maps.append({
            "xt": xt,
            "wq": wq2.astype(np.float32),
            "wk": wk2.astype(np.float32),
            "wv": wv2.astype(np.float32),
            "wot": wot.astype(np.float32),
            "mask": mask,
        })
    return in_maps


_NC = None


def kernel(inputs, Wk, Wq, Wv, Wo, bo):
    global _NC
    if _NC is None:
        _NC = build_nc()
    in_maps = make_in_maps(inputs, Wk, Wq, Wv, Wo)
    res = run_bass_kernel_spmd(_NC, in_maps, core_ids=list(range(N_CORES)))
    z = np.zeros((B, T, E), dtype=np.float32)
    for c in range(N_CORES):
        z += res.results[c]["zp"].astype(np.float32)
    return z + bo.astype(np.float32)
